# revision 56
# baseline (speedup 1.0000x reference)
"""MultiHeadAttention TRN2 Bass kernel.

Problem: B=4, S=2048, D=768, H=12 heads (DK=64).
Sharding: 8 cores = (batch b in 0..3) x (head-half in 0..1); each core
computes 6 heads of one batch element end-to-end (tensor-parallel over
heads within a batch). Host pre-transposes activations to [D, S] (bf16),
slices projection weights per head-half, and sums the two partial
outputs per batch (+ bv@Wo + bo correction, exact because softmax rows
sum to 1). Masked keys are compacted out of k/v on the host (the mask
is per-key) and padded to SKV (multiple of 128, >= 1024); mv marks real
keys and is folded into vh_aug so padding contributes exactly 0 to both
the softmax numerator and denominator.

The scalar-engine exp stream (108 ACTIVATE of [128,1024] at ~1.12us =
~121us total) is the hard wall -- it is the only engine that can run
exp, and the count is sharding-invariant. The schedule keeps it
saturated from as early as possible:

- DMA: few big pieces (the packets of one dma_start spray round-robin
  across all 16 DMA engines, so big pieces lose nothing), issued in
  consumption order over the 3 DMA-capable sequencers (sync, gpsimd,
  scalar). The scalar sequencer gets only the first 6 pieces: later
  issues block on DMA queue-slot waits and would push the first
  ACTIVATE out by ~8us.
- tiles are split by consumer deadline (kta/ktb, qt0/qtr, vtr ranges)
  because a DMA-write -> compute dependency is effectively whole-tile.
- phase 1: all k-proj + v-proj st0-2 + q-proj (dt0, first 512 cols) +
  the two hoisted scores -> first exp at ~31us (vs 34.4 baseline, and
  without the baseline's 7.5us serial-v-proj ACT gap).
- v-proj st3-8 and q-proj dt1/dt2 (first 512 cols) run as the first
  in-loop filler pieces (one per step); attnv(kc) finds vh[kc] ready
  just in time, buffered by an 8-deep pt pool.
- steady state: exp(n); scores(n+2) (DEPTH-2 on 2x[128,1024] PSUM
  tiles); drains at kc 0/1 displace the filler slot (dispatched before
  attnv so the DVE chain starts early); otherwise one filler piece per
  step, psB-allocating pieces only at kc in [4, NKT-cl] so the 4-buf
  psB pool rotation never lands on a live ctx tile; attnv last.
- o-proj per 128-q chunk: the dt0+dt1 matmuls (a1/b1) are queued when
  head 3 of the q-block drains -- one group earlier than head 5 -- so
  only the dt2 matmuls + copies + stores remain after the final drain.
  Stores go out per half ([128,512] + [128,256] bf16) as soon as the
  corresponding PSUM->SBUF copy lands, on the sync/gpsimd queues
  mid-kernel and sync/scalar at the tail (keeps gpsimd's queue clear
  for its teardown drain).
"""

import os
import sys
import types
from contextlib import ExitStack

import ml_dtypes
import numpy as np

import concourse.bacc as bacc
import concourse.bass as bass
import concourse.mybir as mybir
import concourse.tile as tile
from concourse import bass_utils
from concourse.bass import ts, ds

F32 = mybir.dt.float32
F32R = mybir.dt.float32r
BF16 = mybir.dt.bfloat16

D = 768        # model dim
DH = 384       # per-core head dim (6 heads x 64)
HPC = 6        # heads per core
VW = HPC * 65  # vh_aug free width (390)


def build_nc(S=2048, SKV=1152, bf16=True):
    nc = bacc.Bacc("TRN2", target_bir_lowering=False, debug=False)

    MMD = BF16 if bf16 else F32R    # matmul operand dtype
    NKT = SKV // 128                # 128-wide k-tiles
    assert SKV % 128 == 0 and NKT >= 8
    QBW = min(512, S)               # attention q-block width
    NQB = S // QBW                  # q blocks
    NU = S // 512                   # q-proj 512-col chunks
    CWK = next(128 * d for d in (3, 2, 1) if NKT % d == 0)  # k-proj chunk
    NVR = (NKT + 2) // 3            # vt column ranges (3 st each)

    qT = nc.dram_tensor("qT", [D, S], MMD, kind="ExternalInput").ap()
    kT = nc.dram_tensor("kT", [D, SKV], MMD, kind="ExternalInput").ap()
    vT = nc.dram_tensor("vT", [D, SKV], MMD, kind="ExternalInput").ap()
    wq = nc.dram_tensor("wq", [D, DH], MMD, kind="ExternalInput").ap()
    wk = nc.dram_tensor("wk", [D, DH], MMD, kind="ExternalInput").ap()
    wv = nc.dram_tensor("wv", [D, DH], MMD, kind="ExternalInput").ap()
    wo = nc.dram_tensor("wo", [DH, D], MMD, kind="ExternalInput").ap()
    # col 0..2 = bq (3 dt-tiles), 3..5 = bk, 6..6+NKT = mv (padding flag)
    smalls = nc.dram_tensor("smalls", [128, 6 + NKT], F32, kind="ExternalInput").ap()
    out = nc.dram_tensor("out", [S, D], BF16, kind="ExternalOutput").ap()

    with tile.TileContext(nc) as tc, ExitStack() as ctx:
        P = 128
        wpool = ctx.enter_context(tc.tile_pool(name="w", bufs=1))
        persist = ctx.enter_context(tc.tile_pool(name="persist", bufs=1))
        ppool = ctx.enter_context(tc.tile_pool(name="p", bufs=8))
        small = ctx.enter_context(tc.tile_pool(name="small", bufs=3))
        outp = ctx.enter_context(tc.tile_pool(name="outp", bufs=2))
        psA = ctx.enter_context(tc.tile_pool(name="psA", bufs=2, space="PSUM"))
        psB = ctx.enter_context(tc.tile_pool(name="psB", bufs=4, space="PSUM"))

        # ---- DMA issue: big pieces, consumption order, 3 sequencers ----
        dmaq = [nc.sync, nc.gpsimd, nc.scalar]
        dqi = [0]

        def dq_start(dst, src):
            dmaq[dqi[0] % len(dmaq)].dma_start(dst, src)
            dqi[0] += 1

        sm_sb = wpool.tile([128, 6 + NKT], F32, name="sm_sb", tag="smalls")
        nc.sync.dma_start(sm_sb[:], smalls[:, :])
        # wave 1: wk + kta (k-proj sc0) interleaved, then ktb
        wk_sb = [wpool.tile([P, DH], MMD, name=f"wk{c}", tag=f"wk{c}") for c in range(6)]
        kta = [persist.tile([P, CWK], MMD, name=f"kta{c}", tag=f"kta{c}") for c in range(6)]
        ktb = [persist.tile([P, SKV - CWK], MMD, name=f"ktb{c}", tag=f"ktb{c}") for c in range(6)]
        for c in range(6):
            dq_start(wk_sb[c][:], wk[ts(c, P), :])
            dq_start(kta[c][:], kT[ts(c, P), 0:CWK])
            if c == 2:
                dmaq.pop()  # scalar sequencer: first 6 issues only
        for c in range(6):
            dq_start(ktb[c][:], kT[ts(c, P), CWK:SKV])

        def kt_sc(c, sc):
            if sc == 0:
                return kta[c][:]
            return ktb[c][:, ds((sc - 1) * CWK, CWK)]

        # wave 2: wv + vt range 0 (phase-1 v-proj st0-2)
        wv_sb = [wpool.tile([P, DH], MMD, name=f"wv{c}", tag=f"wv{c}") for c in range(6)]
        vtr = [
            [
                persist.tile(
                    [P, min(384, SKV - r * 384)], MMD,
                    name=f"vt{r}_{c}", tag=f"vt{r}_{c}",
                )
                for c in range(6)
            ]
            for r in range(NVR)
        ]
        for c in range(6):
            dq_start(wv_sb[c][:], wv[ts(c, P), :])
            dq_start(vtr[0][c][:], vT[ts(c, P), 0:384])

        def vt_st(c, st):
            return vtr[st // 3][c][:, ts(st % 3, P)]

        # wave 3: wq + qt first 512 cols (gates the first scores)
        wq_sb = [wpool.tile([P, DH], MMD, name=f"wq{c}", tag=f"wq{c}") for c in range(6)]
        qt0 = [persist.tile([P, 512], MMD, name=f"qt0_{c}", tag=f"qt0_{c}") for c in range(6)]
        qtr = [persist.tile([P, S - 512], MMD, name=f"qtr{c}", tag=f"qtr{c}") for c in range(6)]
        for c in range(6):
            dq_start(wq_sb[c][:], wq[ts(c, P), :])
            dq_start(qt0[c][:], qT[ts(c, P), 0:512])

        def qt_u(c, u):
            if u == 0:
                return qt0[c][:]
            return qtr[c][:, ds((u - 1) * 512, 512)]

        # wave 4: vt ranges 1-2; wave 5: rest of qT; wave 6: wo
        for r in range(1, NVR):
            for c in range(6):
                dq_start(
                    vtr[r][c][:],
                    vT[ts(c, P), ds(r * 384, min(384, SKV - r * 384))],
                )
        for c in range(6):
            dq_start(qtr[c][:], qT[ts(c, P), 512:S])
        wo_sb = [wpool.tile([P, D], MMD, name=f"wo{c}", tag=f"wo{c}") for c in range(3)]
        for c in range(3):
            dq_start(wo_sb[c][:], wo[ts(c, P), :])

        bq_sb = [sm_sb[:, t : t + 1] for t in range(3)]
        bk_sb = [sm_sb[:, 3 + t : 4 + t] for t in range(3)]
        mv_sb = [sm_sb[:, 6 + st : 7 + st] for st in range(NKT)]
        ones6 = wpool.tile([P, HPC], F32, name="ones6", tag="ones6")
        nc.vector.memset(ones6[:], 1.0)

        # ---- persistent activations ----
        khT = [persist.tile([P, SKV], MMD, name=f"khT{t}", tag=f"khT{t}") for t in range(3)]
        qhT = [persist.tile([P, S], MMD, name=f"qhT{t}", tag=f"qhT{t}") for t in range(3)]
        vh = [persist.tile([P, VW], MMD, name=f"vh{st}", tag=f"vh{st}") for st in range(NKT)]
        cn = [persist.tile([P, S], MMD, name=f"cn{t}", tag=f"cn{t}") for t in range(3)]

        # ---- phase 1: k-proj (all), v-proj st0-2, q-proj dt0 u0 ----
        for sc in range(SKV // CWK):
            for dt in range(3):
                ps = psA.tile([P, CWK], F32, name="psA", tag="psA")
                for c in range(6):
                    nc.tensor.matmul(
                        ps[:], lhsT=wk_sb[c][:, ts(dt, P)], rhs=kt_sc(c, sc),
                        start=(c == 0), stop=(c == 5),
                    )
                nc.vector.tensor_scalar_add(
                    out=khT[dt][:, ts(sc, CWK)], in0=ps[:], scalar1=bk_sb[dt],
                )

        def vproj_sub(st, pool):
            ps = pool.tile([P, 512], F32, name="vps", tag="psA" if pool is psA else "psB")
            for c in range(6):
                nc.tensor.matmul(
                    ps[:, :DH], lhsT=vt_st(c, st), rhs=wv_sb[c][:],
                    start=(c == 0), stop=(c == 5),
                )
            vh3 = vh[st].rearrange("p (h c) -> p h c", c=65)
            nc.vector.tensor_scalar_mul(
                out=vh3[:, :, 0:64],
                in0=ps[:, :DH].rearrange("p (h c) -> p h c", c=64),
                scalar1=mv_sb[st],
            )
            nc.vector.tensor_scalar_mul(
                out=vh3[:, :, 64:65],
                in0=ones6[:].rearrange("p (h c) -> p h c", c=1),
                scalar1=mv_sb[st],
            )

        for st in range(3):
            vproj_sub(st, psA)

        ps = psA.tile([P, 512], F32, name="psA", tag="psA")
        for c in range(6):
            nc.tensor.matmul(
                ps[:], lhsT=wq_sb[c][:, ts(0, P)], rhs=qt_u(c, 0),
                start=(c == 0), stop=(c == 5),
            )
        nc.vector.tensor_scalar_add(
            out=qhT[0][:, 0:512], in0=ps[:], scalar1=bq_sb[0],
        )

        # ---- phase 2: attention, head-pair steps (baseline discipline) ----
        hq = [(pr, qb) for qb in range(NQB) for pr in range(3)]
        steps = [(pr, qb, kc) for (pr, qb) in hq for kc in range(NKT)]

        ctx_ps = {}
        st_ps = {}

        def scores(pr, qb, kc):
            ps = psA.tile([P, 1024], F32, name="psA", tag="psA")
            for hh in range(2):
                nc.tensor.matmul(
                    ps[:, ts(hh, 512)],
                    lhsT=khT[pr][64 * hh : 64 * hh + 64, ts(kc, P)],
                    rhs=qhT[pr][64 * hh : 64 * hh + 64, ts(qb, QBW)],
                    start=True,
                    stop=True,
                )
            st_ps[(pr, qb, kc)] = ps

        scores(*steps[0])
        scores(*steps[1])

        def attnv(pr, qb, kc, pt):
            for hh in range(2):
                h = 2 * pr + hh
                nc.tensor.matmul(
                    ctx_ps[(h, qb)][0:65, :],
                    lhsT=vh[kc][:, ds(65 * h, 65)],
                    rhs=pt[:, ts(hh, 512)],
                    start=(kc == 0),
                    stop=(kc == NKT - 1),
                )

        def drain(h, qb):
            """Normalize + store ctx for a finished (h, qb)."""
            dt, pb = h // 2, 64 * (h % 2)
            cps = ctx_ps.pop((h, qb))
            den = small.tile([1, QBW], F32, name="den", tag="den")
            nc.vector.tensor_copy(den[:], cps[64:65, :])
            rs = small.tile([1, QBW], F32, name="rs", tag="rs")
            nc.vector.reciprocal_approx_fast(rs[:], den[:])
            bcs = small.tile([64, QBW], F32, name="bcs", tag="bcs")
            nc.gpsimd.partition_broadcast(bcs[:], rs[:])
            if pb == 0:
                nc.vector.tensor_tensor(
                    out=cn[dt][0:64, ts(qb, QBW)],
                    in0=cps[0:64, :],
                    in1=bcs[:],
                    op=mybir.AluOpType.mult,
                )
            else:
                tmp = small.tile([64, QBW], MMD, name="tmp", tag="tmp")
                nc.vector.tensor_tensor(
                    out=tmp[:], in0=cps[0:64, :], in1=bcs[:],
                    op=mybir.AluOpType.mult,
                )
                nc.sync.dma_start(cn[dt][64:128, ts(qb, QBW)], tmp[:])

        oq = [nc.sync]

        # Fillers are (allocates_psB, chain_len, fn) micro pieces, one per
        # step; allocations only at kc in [4, NKT-cl] (after the previous
        # group's ctx tiles are released) — except group 0 (no prior group).
        pend_fill = []

        def queue_vproj(st):
            pend_fill.append((True, 1, lambda: vproj_sub(st, psB)))

        def queue_qproj(dt, u):
            box = {}

            def p1():
                box["ps"] = psB.tile([P, 512], F32, name="psB", tag="psB")
                for c in range(3):
                    nc.tensor.matmul(
                        box["ps"][:],
                        lhsT=wq_sb[c][:, ts(dt, P)],
                        rhs=qt_u(c, u),
                        start=(c == 0),
                        stop=False,
                    )

            def p2():
                for c in range(3, 6):
                    nc.tensor.matmul(
                        box["ps"][:],
                        lhsT=wq_sb[c][:, ts(dt, P)],
                        rhs=qt_u(c, u),
                        start=False,
                        stop=(c == 5),
                    )
                nc.vector.tensor_scalar_add(
                    out=qhT[dt][:, ds(u * 512, 512)],
                    in0=box["ps"][:], scalar1=bq_sb[dt],
                )

            pend_fill.append((True, 2, p1))
            pend_fill.append((False, 0, p2))

        obox = {}

        def queue_oproj_ab1(qc):
            """dt0+dt1 o-proj matmuls — ready once heads 0-3 drained."""
            box = obox.setdefault(qc, {})

            def a1():
                box["ups"] = psB.tile([P, 512], F32, name="psB", tag="psB")
                for dt in range(2):
                    nc.tensor.matmul(
                        box["ups"][:],
                        lhsT=cn[dt][:, ts(qc, P)],
                        rhs=wo_sb[dt][:, ds(0, 512)],
                        start=(dt == 0),
                        stop=False,
                    )

            def b1():
                box["ups2"] = psB.tile([P, 256], F32, name="psB2", tag="psB")
                for dt in range(2):
                    nc.tensor.matmul(
                        box["ups2"][:, 0:256],
                        lhsT=cn[dt][:, ts(qc, P)],
                        rhs=wo_sb[dt][:, ds(512, 256)],
                        start=(dt == 0),
                        stop=False,
                    )

            pend_fill.append((True, 1, a1))
            pend_fill.append((True, 1, b1))

        def queue_oproj_ab2(qc):
            """dt2 matmuls + copies + stores — after heads 4/5 drain."""
            box = obox[qc]

            def a2():
                nc.tensor.matmul(
                    box["ups"][:],
                    lhsT=cn[2][:, ts(qc, P)],
                    rhs=wo_sb[2][:, ds(0, 512)],
                    start=False,
                    stop=True,
                )
                box["ot"] = outp.tile([P, D], MMD, name="ot", tag="ot")
                nc.vector.tensor_copy(box["ot"][:, 0:512], box["ups"][:, 0:512])
                oq[qc % len(oq)].dma_start(
                    out[ts(qc, P), 0:512], box["ot"][:, 0:512]
                )

            def b2():
                nc.tensor.matmul(
                    box["ups2"][:, 0:256],
                    lhsT=cn[2][:, ts(qc, P)],
                    rhs=wo_sb[2][:, ds(512, 256)],
                    start=False,
                    stop=True,
                )
                nc.vector.tensor_copy(box["ot"][:, 512:768], box["ups2"][:, 0:256])
                oq[(qc + 1) % len(oq)].dma_start(
                    out[ts(qc, P), 512:768], box["ot"][:, 512:768]
                )
                obox.pop(qc)

            pend_fill.append((False, 0, a2))
            pend_fill.append((False, 0, b2))

        # initial fillers: v-proj st3-8 (one whole chain per step, needed by
        # attnv at kc=st), then q-proj dt1/dt2 for the first 512 cols, then
        # the remaining q-proj chunks.
        for st in range(3, 6):
            queue_vproj(st)
        queue_qproj(1, 0)
        for st in range(6, NKT):
            queue_vproj(st)
        queue_qproj(2, 0)
        for u in range(1, NU):
            for dt in range(3):
                queue_qproj(dt, u)

        DEPTH = 2
        pend_drain = []
        for n, (pr, qb, kc) in enumerate(steps):
            if kc == 0:
                for hh in range(2):
                    ctx_ps[(2 * pr + hh, qb)] = psB.tile(
                        [P, QBW], F32, name="psB", tag="psB"
                    )
            pt = ppool.tile([P, 1024], MMD, name="pt", tag="pt")
            nc.scalar.activation(
                pt[:], st_ps.pop((pr, qb, kc))[:],
                mybir.ActivationFunctionType.Exp, scale=0.125,
            )
            if n + DEPTH < len(steps):
                scores(*steps[n + DEPTH])
            if kc in (0, 1) and pend_drain:
                hd, qd = pend_drain.pop(0)
                drain(hd, qd)
                if hd == 3:
                    for qcx in range(qd * (QBW // P), (qd + 1) * (QBW // P)):
                        queue_oproj_ab1(qcx)
                elif hd == HPC - 1:
                    for qcx in range(qd * (QBW // P), (qd + 1) * (QBW // P)):
                        queue_oproj_ab2(qcx)
            elif pend_fill:
                na, cl, fn = pend_fill[0]
                if (not na) or (n < 4) or (4 <= kc <= NKT - cl) or (
                    n >= len(steps) - 8
                ):
                    pend_fill.pop(0)
                    fn()
            attnv(pr, qb, kc, pt)
            if kc == NKT - 1:
                pend_drain.extend([(2 * pr, qb), (2 * pr + 1, qb)])
        while pend_fill:
            pend_fill.pop(0)[2]()
        # small keep-warm bridge before the final drains
        wps = psA.tile([P, 512], F32, name="psA", tag="psA")
        for _ in range(4):
            nc.tensor.matmul(
                wps[:], lhsT=khT[0][:, 0:128], rhs=khT[0][:, 0:512],
                start=True, stop=True,
            )
        # batch the final drains phase-by-phase so the two DVE chains
        # and the two gpsimd broadcasts interleave instead of serializing
        infos = []
        for hd, qd in pend_drain:
            dt, pb = hd // 2, 64 * (hd % 2)
            cps = ctx_ps.pop((hd, qd))
            den = small.tile([1, QBW], F32, name="den", tag="den")
            nc.vector.tensor_copy(den[:], cps[64:65, :])
            rs = small.tile([1, QBW], F32, name="rs", tag="rs")
            nc.vector.reciprocal_approx_fast(rs[:], den[:])
            bcs = small.tile([64, QBW], F32, name="bcs", tag="bcs")
            nc.gpsimd.partition_broadcast(bcs[:], rs[:])
            infos.append((hd, qd, dt, pb, cps, bcs[:]))
        for hd, qd, dt, pb, cps, bcs in infos:
            if pb == 0:
                nc.vector.tensor_tensor(
                    out=cn[dt][0:64, ts(qd, QBW)],
                    in0=cps[0:64, :], in1=bcs,
                    op=mybir.AluOpType.mult,
                )
            else:
                tmp = small.tile([64, QBW], MMD, name="tmp", tag="tmp")
                nc.vector.tensor_tensor(
                    out=tmp[:], in0=cps[0:64, :], in1=bcs,
                    op=mybir.AluOpType.mult,
                )
                nc.sync.dma_start(cn[dt][64:128, ts(qd, QBW)], tmp[:])
            if hd == 3:
                for qcx in range(qd * (QBW // P), (qd + 1) * (QBW // P)):
                    queue_oproj_ab1(qcx)
            elif hd == HPC - 1:
                for qcx in range(qd * (QBW // P), (qd + 1) * (QBW // P)):
                    queue_oproj_ab2(qcx)
        oq[:] = [nc.sync, nc.scalar]  # keep the tail off gpsimd's queue
        while pend_fill:
            pend_fill.pop(0)[2]()

    nc.compile()
    return nc


_NC_CACHE = {}


def _get_nc(S, SKV, bf16=True):
    key = (S, SKV, bf16)
    if key not in _NC_CACHE:
        _NC_CACHE[key] = build_nc(S, SKV, bf16)
    return _NC_CACHE[key]


def _install_ntff_hook():
    try:
        mod = types.ModuleType("antenv.axon_hooks")
        state = {"hook": None}
        mod.set_axon_ntff_profile_hook = lambda h: state.__setitem__("hook", h)
        mod.get_axon_ntff_profile_hook = lambda: state["hook"]
        sys.modules["antenv.axon_hooks"] = mod
        from trn_agent_boot.trn_boot import _ntff_profile_via_ctypes

        mod.set_axon_ntff_profile_hook(
            _ntff_profile_via_ctypes("/opt/axon/libaxon_pjrt.so")
        )
        bass_utils.upload_artifacts = lambda tmpdir: "local://" + tmpdir
        return state["hook"] is not None
    except Exception:
        return False


def run_cores(in_maps, S=2048, SKV=1152, bf16=True, profile=False):
    nc = _get_nc(S, SKV, bf16)
    trace = bool(profile) and _install_ntff_hook()
    res = bass_utils.run_bass_kernel_spmd(
        nc, in_maps, core_ids=list(range(len(in_maps))), trace=trace
    )
    return res


def make_in_maps(q, k, v, mask, Wq, bq, Wk, bk, Wv, Wo, bf16=True):
    B, S, _ = q.shape
    mmd = ml_dtypes.bfloat16 if bf16 else np.float32
    q = np.asarray(q, np.float32)
    k = np.asarray(k, np.float32)
    v = np.asarray(v, np.float32)
    keep = ~np.asarray(mask).reshape(B, S)
    counts = keep.sum(axis=1)
    SKV = max(1024, int(-(-int(counts.max()) // 128)) * 128)
    Wq, Wk, Wv, Wo = (np.asarray(a, np.float32) for a in (Wq, Wk, Wv, Wo))
    bq, bk = np.asarray(bq, np.float32), np.asarray(bk, np.float32)
    in_maps = []
    NKT = SKV // 128
    for b in range(B):
        idx = np.nonzero(keep[b])[0]
        n = len(idx)
        kTc = np.zeros((D, SKV), np.float32)
        kTc[:, :n] = k[b][idx].T
        vTc = np.zeros((D, SKV), np.float32)
        vTc[:, :n] = v[b][idx].T
        mvec = np.zeros(SKV, np.float32)
        mvec[:n] = 1.0
        qTb = np.ascontiguousarray(q[b].T).astype(mmd)
        kTc = kTc.astype(mmd)
        vTc = vTc.astype(mmd)
        for half in range(2):
            hs = slice(DH * half, DH * (half + 1))
            sm = np.zeros((128, 6 + NKT), np.float32)
            sm[:, 0:3] = bq[hs].reshape(3, 128).T
            sm[:, 3:6] = bk[hs].reshape(3, 128).T
            sm[:, 6:] = mvec.reshape(NKT, 128).T
            in_maps.append(
                {
                    "qT": qTb,
                    "kT": kTc,
                    "vT": vTc,
                    "wq": np.ascontiguousarray(Wq[:, hs]).astype(mmd),
                    "wk": np.ascontiguousarray(Wk[:, hs]).astype(mmd),
                    "wv": np.ascontiguousarray(Wv[:, hs]).astype(mmd),
                    "wo": np.ascontiguousarray(Wo[hs, :]).astype(mmd),
                    "smalls": sm,
                }
            )
    return in_maps, SKV


def kernel(q, k, v, mask, Wq, bq, Wk, bk, Wv, bv, Wo, bo):
    q = np.asarray(q, np.float32)
    B, S, _ = q.shape
    bf16 = os.environ.get("BASS_PRECISE") != "1"
    in_maps, SKV = make_in_maps(q, k, v, mask, Wq, bq, Wk, bk, Wv, Wo, bf16=bf16)
    res = run_cores(
        in_maps, S=S, SKV=SKV, bf16=bf16,
        profile=os.environ.get("BASS_PROFILE") == "1",
    )
    if os.environ.get("BASS_PROFILE") == "1" and res.exec_time_ns is not None:
        print(f"HW exec time: {res.exec_time_ns} ns")
    cvec = (
        np.asarray(bv, np.float32) @ np.asarray(Wo, np.float32)
        + np.asarray(bo, np.float32)
    )
    out = np.empty((B, S, D), np.float32)
    for b in range(B):
        out[b] = (
            np.asarray(res.results[2 * b]["out"], np.float32)
            + np.asarray(res.results[2 * b + 1]["out"], np.float32)
            + cvec
        )
    return out


# revision 57
# speedup vs baseline: 1.0137x; 1.0137x over previous
"""MultiHeadAttention TRN2 Bass kernel.

Problem: B=4, S=2048, D=768, H=12 heads (DK=64).
Sharding: 8 cores = (batch b in 0..3) x (head-half in 0..1); each core
computes 6 heads of one batch element end-to-end (tensor-parallel over
heads within a batch). Host pre-transposes activations to [D, S] (bf16),
slices projection weights per head-half, and sums the two partial
outputs per batch (+ bv@Wo + bo correction, exact because softmax rows
sum to 1). Masked keys are compacted out of k/v on the host (the mask
is per-key) and padded to SKV (multiple of 128, >= 1024); mv marks real
keys and is folded into vh_aug so padding contributes exactly 0 to both
the softmax numerator and denominator.

The scalar-engine exp stream (108 ACTIVATE of [128,1024] at ~1.12us =
~121us total) is the hard wall -- it is the only engine that can run
exp, and the count is sharding-invariant. The schedule keeps it
saturated from as early as possible:

- DMA: few big pieces (the packets of one dma_start spray round-robin
  across all 16 DMA engines, so big pieces lose nothing), issued in
  consumption order over the 3 DMA-capable sequencers (sync, gpsimd,
  scalar). The scalar sequencer gets only the first 6 pieces: later
  issues block on DMA queue-slot waits and would push the first
  ACTIVATE out by ~8us.
- tiles are split by consumer deadline (kta/ktb, qt0/qtr, vtr ranges)
  because a DMA-write -> compute dependency is effectively whole-tile.
- phase 1: all k-proj + v-proj st0-2 + q-proj (dt0, first 512 cols) +
  the two hoisted scores -> first exp at ~31us (vs 34.4 baseline, and
  without the baseline's 7.5us serial-v-proj ACT gap).
- v-proj st3-8 and q-proj dt1/dt2 (first 512 cols) run as the first
  in-loop filler pieces (one per step); attnv(kc) finds vh[kc] ready
  just in time, buffered by an 8-deep pt pool.
- steady state: exp(n); scores(n+2) (DEPTH-2 on 2x[128,1024] PSUM
  tiles); drains at kc 0/1 displace the filler slot (dispatched before
  attnv so the DVE chain starts early); otherwise one filler piece per
  step, psB-allocating pieces only at kc in [4, NKT-cl] so the 4-buf
  psB pool rotation never lands on a live ctx tile; attnv last.
- o-proj per 128-q chunk: the dt0+dt1 matmuls (a1/b1) are queued when
  head 3 of the q-block drains -- one group earlier than head 5 -- so
  only the dt2 matmuls + copies + stores remain after the final drain.
  Stores go out per half ([128,512] + [128,256] bf16) as soon as the
  corresponding PSUM->SBUF copy lands, on the sync/gpsimd queues
  mid-kernel and sync/scalar at the tail (keeps gpsimd's queue clear
  for its teardown drain).
"""

import os
import sys
import types
from contextlib import ExitStack

import ml_dtypes
import numpy as np

import concourse.bacc as bacc
import concourse.bass as bass
import concourse.mybir as mybir
import concourse.tile as tile
from concourse import bass_utils
from concourse.bass import ts, ds

F32 = mybir.dt.float32
F32R = mybir.dt.float32r
BF16 = mybir.dt.bfloat16

D = 768        # model dim
DH = 384       # per-core head dim (6 heads x 64)
HPC = 6        # heads per core
VW = HPC * 65  # vh_aug free width (390)


def build_nc(S=2048, SKV=1152, bf16=True):
    nc = bacc.Bacc("TRN2", target_bir_lowering=False, debug=False)

    MMD = BF16 if bf16 else F32R    # matmul operand dtype
    NKT = SKV // 128                # 128-wide k-tiles
    assert SKV % 128 == 0 and NKT >= 8
    QBW = min(512, S)               # attention q-block width
    NQB = S // QBW                  # q blocks
    NU = S // 512                   # q-proj 512-col chunks
    CWK = next(128 * d for d in (3, 2, 1) if NKT % d == 0)  # k-proj chunk
    NVR = (NKT + 2) // 3            # vt column ranges (3 st each)

    qT = nc.dram_tensor("qT", [D, S], MMD, kind="ExternalInput").ap()
    kT = nc.dram_tensor("kT", [D, SKV], MMD, kind="ExternalInput").ap()
    vT = nc.dram_tensor("vT", [D, SKV], MMD, kind="ExternalInput").ap()
    wq = nc.dram_tensor("wq", [D, DH], MMD, kind="ExternalInput").ap()
    wk = nc.dram_tensor("wk", [D, DH], MMD, kind="ExternalInput").ap()
    wv = nc.dram_tensor("wv", [D, DH], MMD, kind="ExternalInput").ap()
    wo = nc.dram_tensor("wo", [DH, D], MMD, kind="ExternalInput").ap()
    # col 0..2 = bq (3 dt-tiles), 3..5 = bk, 6..6+NKT = mv (padding flag)
    smalls = nc.dram_tensor("smalls", [128, 6 + NKT], F32, kind="ExternalInput").ap()
    out = nc.dram_tensor("out", [S, D], BF16, kind="ExternalOutput").ap()

    with tile.TileContext(nc) as tc, ExitStack() as ctx:
        P = 128
        wpool = ctx.enter_context(tc.tile_pool(name="w", bufs=1))
        persist = ctx.enter_context(tc.tile_pool(name="persist", bufs=1))
        ppool = ctx.enter_context(tc.tile_pool(name="p", bufs=8))
        small = ctx.enter_context(tc.tile_pool(name="small", bufs=3))
        outp = ctx.enter_context(tc.tile_pool(name="outp", bufs=2))
        psA = ctx.enter_context(tc.tile_pool(name="psA", bufs=2, space="PSUM"))
        psB = ctx.enter_context(tc.tile_pool(name="psB", bufs=4, space="PSUM"))

        # ---- DMA issue: big pieces, consumption order, 3 sequencers ----
        dmaq = [nc.sync, nc.gpsimd, nc.scalar]
        dqi = [0]

        def dq_start(dst, src):
            dmaq[dqi[0] % len(dmaq)].dma_start(dst, src)
            dqi[0] += 1

        sm_sb = wpool.tile([128, 6 + NKT], F32, name="sm_sb", tag="smalls")
        nc.sync.dma_start(sm_sb[:], smalls[:, :])
        # wave 1: wk + kta (k-proj sc0) interleaved, then ktb
        wk_sb = [wpool.tile([P, DH], MMD, name=f"wk{c}", tag=f"wk{c}") for c in range(6)]
        kta = [persist.tile([P, CWK], MMD, name=f"kta{c}", tag=f"kta{c}") for c in range(6)]
        ktb = [persist.tile([P, SKV - CWK], MMD, name=f"ktb{c}", tag=f"ktb{c}") for c in range(6)]
        for c in range(6):
            dq_start(wk_sb[c][:], wk[ts(c, P), :])
            dq_start(kta[c][:], kT[ts(c, P), 0:CWK])
            if c == 2:
                dmaq.pop()  # scalar sequencer: first 6 issues only
        for c in range(6):
            dq_start(ktb[c][:], kT[ts(c, P), CWK:SKV])

        def kt_sc(c, sc):
            if sc == 0:
                return kta[c][:]
            return ktb[c][:, ds((sc - 1) * CWK, CWK)]

        # wave 2: wv + vt range 0 (phase-1 v-proj st0-2)
        wv_sb = [wpool.tile([P, DH], MMD, name=f"wv{c}", tag=f"wv{c}") for c in range(6)]
        vtr = [
            [
                persist.tile(
                    [P, min(384, SKV - r * 384)], MMD,
                    name=f"vt{r}_{c}", tag=f"vt{r}_{c}",
                )
                for c in range(6)
            ]
            for r in range(NVR)
        ]
        for c in range(6):
            dq_start(wv_sb[c][:], wv[ts(c, P), :])
            dq_start(vtr[0][c][:], vT[ts(c, P), 0:384])

        def vt_st(c, st):
            return vtr[st // 3][c][:, ts(st % 3, P)]

        # wave 3: wq + qt first 512 cols (gates the first scores)
        wq_sb = [wpool.tile([P, DH], MMD, name=f"wq{c}", tag=f"wq{c}") for c in range(6)]
        qt0 = [persist.tile([P, 512], MMD, name=f"qt0_{c}", tag=f"qt0_{c}") for c in range(6)]
        qtr = [persist.tile([P, S - 512], MMD, name=f"qtr{c}", tag=f"qtr{c}") for c in range(6)]
        for c in range(6):
            dq_start(wq_sb[c][:], wq[ts(c, P), :])
            dq_start(qt0[c][:], qT[ts(c, P), 0:512])

        def qt_u(c, u):
            if u == 0:
                return qt0[c][:]
            return qtr[c][:, ds((u - 1) * 512, 512)]

        # wave 4: vt ranges 1-2; wave 5: rest of qT; wave 6: wo
        for r in range(1, NVR):
            for c in range(6):
                dq_start(
                    vtr[r][c][:],
                    vT[ts(c, P), ds(r * 384, min(384, SKV - r * 384))],
                )
        for c in range(6):
            dq_start(qtr[c][:], qT[ts(c, P), 512:S])
        wo_sb = [wpool.tile([P, D], MMD, name=f"wo{c}", tag=f"wo{c}") for c in range(3)]
        for c in range(3):
            dq_start(wo_sb[c][:], wo[ts(c, P), :])

        bq_sb = [sm_sb[:, t : t + 1] for t in range(3)]
        bk_sb = [sm_sb[:, 3 + t : 4 + t] for t in range(3)]
        mv_sb = [sm_sb[:, 6 + st : 7 + st] for st in range(NKT)]
        ones6 = wpool.tile([P, HPC], F32, name="ones6", tag="ones6")
        nc.vector.memset(ones6[:], 1.0)

        # ---- persistent activations ----
        khT = [persist.tile([P, SKV], MMD, name=f"khT{t}", tag=f"khT{t}") for t in range(3)]
        qhT = [persist.tile([P, S], MMD, name=f"qhT{t}", tag=f"qhT{t}") for t in range(3)]
        vh = [persist.tile([P, VW], MMD, name=f"vh{st}", tag=f"vh{st}") for st in range(NKT)]
        cn = [persist.tile([P, S], MMD, name=f"cn{t}", tag=f"cn{t}") for t in range(3)]

        # ---- phase 1: k-proj (all), v-proj st0-2, q-proj dt0 u0 ----
        for sc in range(SKV // CWK):
            for dt in range(3):
                ps = psA.tile([P, CWK], F32, name="psA", tag="psA")
                for c in range(6):
                    nc.tensor.matmul(
                        ps[:], lhsT=wk_sb[c][:, ts(dt, P)], rhs=kt_sc(c, sc),
                        start=(c == 0), stop=(c == 5),
                    )
                nc.vector.tensor_scalar_add(
                    out=khT[dt][:, ts(sc, CWK)], in0=ps[:], scalar1=bk_sb[dt],
                )

        def vproj_sub(st, pool):
            ps = pool.tile([P, 512], F32, name="vps", tag="psA" if pool is psA else "psB")
            for c in range(6):
                nc.tensor.matmul(
                    ps[:, :DH], lhsT=vt_st(c, st), rhs=wv_sb[c][:],
                    start=(c == 0), stop=(c == 5),
                )
            vh3 = vh[st].rearrange("p (h c) -> p h c", c=65)
            nc.vector.tensor_scalar_mul(
                out=vh3[:, :, 0:64],
                in0=ps[:, :DH].rearrange("p (h c) -> p h c", c=64),
                scalar1=mv_sb[st],
            )
            nc.vector.tensor_scalar_mul(
                out=vh3[:, :, 64:65],
                in0=ones6[:].rearrange("p (h c) -> p h c", c=1),
                scalar1=mv_sb[st],
            )

        for st in range(3):
            vproj_sub(st, psA)

        ps = psA.tile([P, 512], F32, name="psA", tag="psA")
        for c in range(6):
            nc.tensor.matmul(
                ps[:], lhsT=wq_sb[c][:, ts(0, P)], rhs=qt_u(c, 0),
                start=(c == 0), stop=(c == 5),
            )
        nc.vector.tensor_scalar_add(
            out=qhT[0][:, 0:512], in0=ps[:], scalar1=bq_sb[0],
        )

        # ---- phase 2: attention, head-pair steps (baseline discipline) ----
        hq = [(pr, qb) for qb in range(NQB) for pr in range(3)]
        steps = [(pr, qb, kc) for (pr, qb) in hq for kc in range(NKT)]

        ctx_ps = {}
        st_ps = {}

        def scores(pr, qb, kc):
            ps = psA.tile([P, 1024], F32, name="psA", tag="psA")
            for hh in range(2):
                nc.tensor.matmul(
                    ps[:, ts(hh, 512)],
                    lhsT=khT[pr][64 * hh : 64 * hh + 64, ts(kc, P)],
                    rhs=qhT[pr][64 * hh : 64 * hh + 64, ts(qb, QBW)],
                    start=True,
                    stop=True,
                )
            st_ps[(pr, qb, kc)] = ps

        scores(*steps[0])
        scores(*steps[1])

        def attnv(pr, qb, kc, pt):
            for hh in range(2):
                h = 2 * pr + hh
                nc.tensor.matmul(
                    ctx_ps[(h, qb)][0:65, :],
                    lhsT=vh[kc][:, ds(65 * h, 65)],
                    rhs=pt[:, ts(hh, 512)],
                    start=(kc == 0),
                    stop=(kc == NKT - 1),
                )

        def drain(h, qb):
            """Normalize + store ctx for a finished (h, qb)."""
            dt, pb = h // 2, 64 * (h % 2)
            cps = ctx_ps.pop((h, qb))
            den = small.tile([1, QBW], F32, name="den", tag="den")
            nc.vector.tensor_copy(den[:], cps[64:65, :])
            rs = small.tile([1, QBW], F32, name="rs", tag="rs")
            nc.vector.reciprocal_approx_fast(rs[:], den[:])
            bcs = small.tile([64, QBW], F32, name="bcs", tag="bcs")
            nc.gpsimd.partition_broadcast(bcs[:], rs[:])
            if pb == 0:
                nc.vector.tensor_tensor(
                    out=cn[dt][0:64, ts(qb, QBW)],
                    in0=cps[0:64, :],
                    in1=bcs[:],
                    op=mybir.AluOpType.mult,
                )
            else:
                tmp = small.tile([64, QBW], MMD, name="tmp", tag="tmp")
                nc.vector.tensor_tensor(
                    out=tmp[:], in0=cps[0:64, :], in1=bcs[:],
                    op=mybir.AluOpType.mult,
                )
                nc.sync.dma_start(cn[dt][64:128, ts(qb, QBW)], tmp[:])

        oq = [nc.sync, nc.gpsimd]

        # Fillers are (allocates_psB, chain_len, fn) micro pieces, one per
        # step; allocations only at kc in [4, NKT-cl] (after the previous
        # group's ctx tiles are released) — except group 0 (no prior group).
        pend_fill = []

        def queue_vproj(st):
            pend_fill.append((True, 1, lambda: vproj_sub(st, psB)))

        def queue_qproj(dt, u):
            box = {}

            def p1():
                box["ps"] = psB.tile([P, 512], F32, name="psB", tag="psB")
                for c in range(3):
                    nc.tensor.matmul(
                        box["ps"][:],
                        lhsT=wq_sb[c][:, ts(dt, P)],
                        rhs=qt_u(c, u),
                        start=(c == 0),
                        stop=False,
                    )

            def p2():
                for c in range(3, 6):
                    nc.tensor.matmul(
                        box["ps"][:],
                        lhsT=wq_sb[c][:, ts(dt, P)],
                        rhs=qt_u(c, u),
                        start=False,
                        stop=(c == 5),
                    )
                nc.vector.tensor_scalar_add(
                    out=qhT[dt][:, ds(u * 512, 512)],
                    in0=box["ps"][:], scalar1=bq_sb[dt],
                )

            pend_fill.append((True, 2, p1))
            pend_fill.append((False, 0, p2))

        obox = {}

        def queue_oproj_ab1(qc):
            """dt0+dt1 o-proj matmuls — ready once heads 0-3 drained."""
            box = obox.setdefault(qc, {})

            def a1():
                box["ups"] = psB.tile([P, 512], F32, name="psB", tag="psB")
                for dt in range(2):
                    nc.tensor.matmul(
                        box["ups"][:],
                        lhsT=cn[dt][:, ts(qc, P)],
                        rhs=wo_sb[dt][:, ds(0, 512)],
                        start=(dt == 0),
                        stop=False,
                    )

            def b1():
                box["ups2"] = psB.tile([P, 256], F32, name="psB2", tag="psB")
                for dt in range(2):
                    nc.tensor.matmul(
                        box["ups2"][:, 0:256],
                        lhsT=cn[dt][:, ts(qc, P)],
                        rhs=wo_sb[dt][:, ds(512, 256)],
                        start=(dt == 0),
                        stop=False,
                    )

            pend_fill.append((True, 1, a1))
            pend_fill.append((True, 1, b1))

        def queue_oproj_ab2(qc):
            """dt2 matmuls + copies + stores — after heads 4/5 drain."""
            box = obox[qc]

            def a2():
                nc.tensor.matmul(
                    box["ups"][:],
                    lhsT=cn[2][:, ts(qc, P)],
                    rhs=wo_sb[2][:, ds(0, 512)],
                    start=False,
                    stop=True,
                )
                box["ot"] = outp.tile([P, D], MMD, name="ot", tag="ot")
                nc.vector.tensor_copy(box["ot"][:, 0:512], box["ups"][:, 0:512])
                oq[qc % len(oq)].dma_start(
                    out[ts(qc, P), 0:512], box["ot"][:, 0:512]
                )

            def b2():
                nc.tensor.matmul(
                    box["ups2"][:, 0:256],
                    lhsT=cn[2][:, ts(qc, P)],
                    rhs=wo_sb[2][:, ds(512, 256)],
                    start=False,
                    stop=True,
                )
                nc.vector.tensor_copy(box["ot"][:, 512:768], box["ups2"][:, 0:256])
                oq[(qc + 1) % len(oq)].dma_start(
                    out[ts(qc, P), 512:768], box["ot"][:, 512:768]
                )
                obox.pop(qc)

            pend_fill.append((False, 0, a2))
            pend_fill.append((False, 0, b2))

        # initial fillers: v-proj st3-8 (one whole chain per step, needed by
        # attnv at kc=st), then q-proj dt1/dt2 for the first 512 cols, then
        # the remaining q-proj chunks.
        for st in range(3, 6):
            queue_vproj(st)
        queue_qproj(1, 0)
        for st in range(6, NKT):
            queue_vproj(st)
        queue_qproj(2, 0)
        for u in range(1, NU):
            for dt in range(3):
                queue_qproj(dt, u)

        DEPTH = 2
        pend_drain = []
        for n, (pr, qb, kc) in enumerate(steps):
            if kc == 0:
                for hh in range(2):
                    ctx_ps[(2 * pr + hh, qb)] = psB.tile(
                        [P, QBW], F32, name="psB", tag="psB"
                    )
            pt = ppool.tile([P, 1024], MMD, name="pt", tag="pt")
            nc.scalar.activation(
                pt[:], st_ps.pop((pr, qb, kc))[:],
                mybir.ActivationFunctionType.Exp, scale=0.125,
            )
            if n + DEPTH < len(steps):
                scores(*steps[n + DEPTH])
            if kc in (0, 1) and pend_drain:
                hd, qd = pend_drain.pop(0)
                drain(hd, qd)
                if hd == 3:
                    for qcx in range(qd * (QBW // P), (qd + 1) * (QBW // P)):
                        queue_oproj_ab1(qcx)
                elif hd == HPC - 1:
                    for qcx in range(qd * (QBW // P), (qd + 1) * (QBW // P)):
                        queue_oproj_ab2(qcx)
            elif pend_fill:
                na, cl, fn = pend_fill[0]
                if (not na) or (n < 4) or (4 <= kc <= NKT - cl) or (
                    n >= len(steps) - 8
                ):
                    pend_fill.pop(0)
                    fn()
            attnv(pr, qb, kc, pt)
            if kc == NKT - 1:
                pend_drain.extend([(2 * pr, qb), (2 * pr + 1, qb)])
        while pend_fill:
            pend_fill.pop(0)[2]()
        # small keep-warm bridge before the final drains
        wps = psA.tile([P, 512], F32, name="psA", tag="psA")
        for _ in range(4):
            nc.tensor.matmul(
                wps[:], lhsT=khT[0][:, 0:128], rhs=khT[0][:, 0:512],
                start=True, stop=True,
            )
        # batch the final drains phase-by-phase so the two DVE chains
        # and the two gpsimd broadcasts interleave instead of serializing
        infos = []
        for hd, qd in pend_drain:
            dt, pb = hd // 2, 64 * (hd % 2)
            cps = ctx_ps.pop((hd, qd))
            den = small.tile([1, QBW], F32, name="den", tag="den")
            nc.vector.tensor_copy(den[:], cps[64:65, :])
            rs = small.tile([1, QBW], F32, name="rs", tag="rs")
            nc.vector.reciprocal_approx_fast(rs[:], den[:])
            bcs = small.tile([64, QBW], F32, name="bcs", tag="bcs")
            nc.gpsimd.partition_broadcast(bcs[:], rs[:])
            infos.append((hd, qd, dt, pb, cps, bcs[:]))
        for hd, qd, dt, pb, cps, bcs in infos:
            if pb == 0:
                nc.vector.tensor_tensor(
                    out=cn[dt][0:64, ts(qd, QBW)],
                    in0=cps[0:64, :], in1=bcs,
                    op=mybir.AluOpType.mult,
                )
            else:
                tmp = small.tile([64, QBW], MMD, name="tmp", tag="tmp")
                nc.vector.tensor_tensor(
                    out=tmp[:], in0=cps[0:64, :], in1=bcs,
                    op=mybir.AluOpType.mult,
                )
                nc.sync.dma_start(cn[dt][64:128, ts(qd, QBW)], tmp[:])
            if hd == 3:
                for qcx in range(qd * (QBW // P), (qd + 1) * (QBW // P)):
                    queue_oproj_ab1(qcx)
            elif hd == HPC - 1:
                for qcx in range(qd * (QBW // P), (qd + 1) * (QBW // P)):
                    queue_oproj_ab2(qcx)
        oq[:] = [nc.sync, nc.scalar]  # keep the tail off gpsimd's queue
        while pend_fill:
            pend_fill.pop(0)[2]()

    nc.compile()
    return nc


_NC_CACHE = {}


def _get_nc(S, SKV, bf16=True):
    key = (S, SKV, bf16)
    if key not in _NC_CACHE:
        _NC_CACHE[key] = build_nc(S, SKV, bf16)
    return _NC_CACHE[key]


def _install_ntff_hook():
    try:
        mod = types.ModuleType("antenv.axon_hooks")
        state = {"hook": None}
        mod.set_axon_ntff_profile_hook = lambda h: state.__setitem__("hook", h)
        mod.get_axon_ntff_profile_hook = lambda: state["hook"]
        sys.modules["antenv.axon_hooks"] = mod
        from trn_agent_boot.trn_boot import _ntff_profile_via_ctypes

        mod.set_axon_ntff_profile_hook(
            _ntff_profile_via_ctypes("/opt/axon/libaxon_pjrt.so")
        )
        bass_utils.upload_artifacts = lambda tmpdir: "local://" + tmpdir
        return state["hook"] is not None
    except Exception:
        return False


def run_cores(in_maps, S=2048, SKV=1152, bf16=True, profile=False):
    nc = _get_nc(S, SKV, bf16)
    trace = bool(profile) and _install_ntff_hook()
    res = bass_utils.run_bass_kernel_spmd(
        nc, in_maps, core_ids=list(range(len(in_maps))), trace=trace
    )
    return res


def make_in_maps(q, k, v, mask, Wq, bq, Wk, bk, Wv, Wo, bf16=True):
    B, S, _ = q.shape
    mmd = ml_dtypes.bfloat16 if bf16 else np.float32
    q = np.asarray(q, np.float32)
    k = np.asarray(k, np.float32)
    v = np.asarray(v, np.float32)
    keep = ~np.asarray(mask).reshape(B, S)
    counts = keep.sum(axis=1)
    SKV = max(1024, int(-(-int(counts.max()) // 128)) * 128)
    Wq, Wk, Wv, Wo = (np.asarray(a, np.float32) for a in (Wq, Wk, Wv, Wo))
    bq, bk = np.asarray(bq, np.float32), np.asarray(bk, np.float32)
    in_maps = []
    NKT = SKV // 128
    for b in range(B):
        idx = np.nonzero(keep[b])[0]
        n = len(idx)
        kTc = np.zeros((D, SKV), np.float32)
        kTc[:, :n] = k[b][idx].T
        vTc = np.zeros((D, SKV), np.float32)
        vTc[:, :n] = v[b][idx].T
        mvec = np.zeros(SKV, np.float32)
        mvec[:n] = 1.0
        qTb = np.ascontiguousarray(q[b].T).astype(mmd)
        kTc = kTc.astype(mmd)
        vTc = vTc.astype(mmd)
        for half in range(2):
            hs = slice(DH * half, DH * (half + 1))
            sm = np.zeros((128, 6 + NKT), np.float32)
            sm[:, 0:3] = bq[hs].reshape(3, 128).T
            sm[:, 3:6] = bk[hs].reshape(3, 128).T
            sm[:, 6:] = mvec.reshape(NKT, 128).T
            in_maps.append(
                {
                    "qT": qTb,
                    "kT": kTc,
                    "vT": vTc,
                    "wq": np.ascontiguousarray(Wq[:, hs]).astype(mmd),
                    "wk": np.ascontiguousarray(Wk[:, hs]).astype(mmd),
                    "wv": np.ascontiguousarray(Wv[:, hs]).astype(mmd),
                    "wo": np.ascontiguousarray(Wo[hs, :]).astype(mmd),
                    "smalls": sm,
                }
            )
    return in_maps, SKV


def kernel(q, k, v, mask, Wq, bq, Wk, bk, Wv, bv, Wo, bo):
    q = np.asarray(q, np.float32)
    B, S, _ = q.shape
    bf16 = os.environ.get("BASS_PRECISE") != "1"
    in_maps, SKV = make_in_maps(q, k, v, mask, Wq, bq, Wk, bk, Wv, Wo, bf16=bf16)
    res = run_cores(
        in_maps, S=S, SKV=SKV, bf16=bf16,
        profile=os.environ.get("BASS_PROFILE") == "1",
    )
    if os.environ.get("BASS_PROFILE") == "1" and res.exec_time_ns is not None:
        print(f"HW exec time: {res.exec_time_ns} ns")
    cvec = (
        np.asarray(bv, np.float32) @ np.asarray(Wo, np.float32)
        + np.asarray(bo, np.float32)
    )
    out = np.empty((B, S, D), np.float32)
    for b in range(B):
        out[b] = (
            np.asarray(res.results[2 * b]["out"], np.float32)
            + np.asarray(res.results[2 * b + 1]["out"], np.float32)
            + cvec
        )
    return out
